# Initial kernel scaffold
#
"""CCALoss (soft-contrastive CLIP + masked BCE + concept-sim KL) on 8 trn2 cores.

Math: with c = (mc==1) binary, jaccard inter = c@cT, union = r_i + r_j -
inter. All three PE contractions (r_j - inter via (1-c)@c, +r_i via
c@ones, inter via c@c) run as fp8 DoubleRow matmuls (K=256 in one pass).
sim5 = 5*inter/union via DVE recip + a fused scalar_tensor_tensor; the
four per-row dots sum_j e^(5sim)*X for X in {img, txt, 5sim, cis} are
fused STT ops with row accumulators. BCE uses the sign trick
bce = ln(1+e^(s*x)), s = mask*(1-2t) in {-1,0,+1}; masked entries each
contribute ln2, subtracted exactly on host. The device ships per-row raw
stats V[128,8] (dots, softmax partition sums Z, bce row sums); the host
does every 1/Z, ln and the final scalar combine in fp64.

Schedule: inputs split into five DMAs on three queues so each consumer's
bytes land just before it runs (sync: fp8 pack in two column chunks so
the a-half matmuls start early; scalar: [cls|cis] halves; gpsimd:
logits). A dependency-free dummy Ln leads the ACT queue so both
ACT_TABLE_LOADs run under the DMA shadow; explicit order deps pin the
ACT sequence exp_sx -> ln -> expQ_a -> expQ_b -> exp_pt. The
union->recip->5sim->exp->dots chain is pipelined in column halves with
per-half accumulator columns in V summed on host.
"""

import os
import numpy as np
import types
from contextlib import ExitStack

import ml_dtypes

import bass_rust as _bass_rust
import concourse.bacc as bacc
import concourse.mybir as mybir
import concourse.tile as tile
from concourse.tile_rust import add_dep_helper
from concourse import bass_utils
from concourse.hw_specs import get_activation_tables

F32 = mybir.dt.float32
BF16 = mybir.dt.bfloat16
FP8 = mybir.dt.float8e4
U32 = mybir.dt.uint32
AF = mybir.ActivationFunctionType
ALU = mybir.AluOpType
AX = mybir.AxisListType

B = 512          # batch
C = 256          # concepts
H = 256          # column half for the pipelined tail
NCORES = 8
BLK = B // NCORES  # 64 rows per core
NST = 8          # stat columns in V

# V column layout ([128, NST]; rows 0:64 / 64:128 hold different stats)
# 0/1: dot e*[img;txt] halves a/b   2/3: dot e*[5sim;cis] halves a/b
# 4: Z of [img;txt]                 5/6: Z of [5sim;cis] halves a/b
# 7: lower only, sum_j ln(1+e^(s*x))
_CACHE = {}

LN2 = float(np.log(2.0))
FP8_ONE_X4 = 0x38383838  # four fp8e4m3 1.0 bytes per uint32


def _patched_act_table_loads(self):
    """Force exp+ln onto the single natural_log_exp_and_others set.

    Same contract as Bacc.insert_act_table_loads: the list index must
    stay aligned with act_info.json (walrus remaps index -> runtime id),
    so sets keep their positions and only lose exp/ln membership.
    """
    has_activation = any(
        isinstance(i, mybir.InstActivation)
        for b in self.main_func.blocks
        for i in b.instructions
    )
    if not has_activation:
        return
    keep = "natural_log_exp_and_others"
    both = {AF.Exp, AF.Ln}
    tables = [
        (name, set(fns) if name == keep else set(fns) - both)
        for name, fns in get_activation_tables(self.m.arch).items()
    ]
    _bass_rust.insert_act_table_loads(self, tables)


def build_nc():
    nc = bacc.Bacc(
        "TRN2", target_bir_lowering=False, debug=False, num_devices=NCORES
    )
    nc.insert_act_table_loads = types.MethodType(_patched_act_table_loads, nc)

    # fp8 packs: [p, two, j] = c^T; A carries batch cols 0:256 plus the
    # stationary blocks ((1-c)^T blk, c^T blk), B carries cols 256:512
    cpa_in = nc.dram_tensor("cpa", [128, 2 * (H + 2 * BLK)], FP8, kind="ExternalInput").ap()
    cpb_in = nc.dram_tensor("cpb", [128, 2 * H], FP8, kind="ExternalInput").ap()
    # [128,512] bf16: rows 0:64 = logits_per_image blk, 64:128 = logits_per_text blk
    pt_in = nc.dram_tensor("pt", [128, B], BF16, kind="ExternalInput").ap()
    # [128,512] f32: rows 0:64 = [concepts_logits blk | s]-as-bf16-bytes (+pad),
    # rows 64:128 = concepts_image_similarity blk
    qc_in = nc.dram_tensor("qcin", [128, B], F32, kind="ExternalInput").ap()
    vout = nc.dram_tensor("vout", [128, NST], F32, kind="ExternalOutput").ap()

    with tile.TileContext(nc) as tc, ExitStack() as ctx:
        pool = ctx.enter_context(tc.tile_pool(name="main", bufs=1))
        psum = ctx.enter_context(tc.tile_pool(name="psum", bufs=1, space="PSUM"))

        cpa = pool.tile([128, 2, H + 2 * BLK], FP8)
        cpb = pool.tile([128, 2, H], FP8)
        ones32 = pool.tile([128, 2 * B // 4], U32)
        PT = pool.tile([128, B], BF16)
        QC = pool.tile([128, B], F32)  # 0:64 = cls bytes then 5sim; 64:128 = cis
        V = pool.tile([128, NST], F32)

        ones = ones32[:].bitcast(FP8).rearrange("p (two w) -> p two w", two=2)
        cls = QC[0:BLK, 0:C].bitcast(BF16)  # [64, 512] bf16: [cl | s]

        # ---- DMA issue spread across engine queues (parallel at t=0) ----
        nc.vector.memset(ones32[:], FP8_ONE_X4)
        nc.vector.memset(V[:], 0.0)
        nc.sync.dma_start(cpa[:], cpa_in[:].rearrange("p (two w) -> p two w", two=2))
        nc.sync.dma_start(cpb[:], cpb_in[:].rearrange("p (two w) -> p two w", two=2))
        nc.scalar.dma_start(QC[:, 0:H], qc_in[:, 0:H])
        nc.scalar.dma_start(QC[:, H:B], qc_in[:, H:B])
        nc.gpsimd.dma_start(PT[:], pt_in[:])

        cfa = cpa[:, :, 0:H]
        onemcb = cpa[:, :, H : H + BLK]
        cblkb = cpa[:, :, H + BLK : H + 2 * BLK]
        cfb = cpb[:, :, 0:H]
        DR = mybir.MatmulPerfMode.DoubleRow

        # ---- jaccard contractions: union and inter, fp8 DoubleRow ----
        pU = [psum.tile([BLK, H], F32, name=f"pU{h}") for h in range(2)]
        pI = [psum.tile([BLK, H], F32, name=f"pI{h}") for h in range(2)]
        for h, cf in enumerate((cfa, cfb)):
            on = ones[:, :, h * H : (h + 1) * H]
            nc.tensor.matmul(pU[h][:], onemcb, cf, start=True, stop=False, perf_mode=DR)
            nc.tensor.matmul(pU[h][:], cblkb, on, start=False, stop=True, perf_mode=DR)
            nc.tensor.matmul(pI[h][:], cblkb, cf, start=True, stop=True, perf_mode=DR)

        # ---- dummy ln: anchors the ACT table load at t~0 (no data deps) ----
        dummy = pool.tile([1, 1], F32)
        i_dummy = nc.scalar.activation(dummy[:], V[0:1, 0:1], AF.Ln, bias=1.0).ins

        # ---- BCE: sx = s*x, then ln(1 + e^sx) row-summed by the ACT accum ----
        sxr = pool.tile([BLK, C], BF16)
        nc.vector.tensor_tensor(sxr[:], cls[:, C : 2 * C], cls[:, 0:C], ALU.mult)
        bexp = pool.tile([BLK, C], BF16)
        i_esx = nc.scalar.activation(bexp[:], sxr[:], AF.Exp).ins
        bln = pool.tile([BLK, C], BF16)  # scrap; accum is the payload
        i_eln = nc.scalar.activation(
            bln[:], bexp[:], AF.Ln, bias=1.0, accum_out=V[0:BLK, 7:8]
        ).ins

        # ---- pipelined halves: recip -> 5sim -> exp -> dup -> dots ----
        urec = pool.tile([BLK, B], F32)
        X = pool.tile([128, B], BF16)       # e^[5sim; cis]
        scrapP = pool.tile([128, B], BF16)
        scrapQ = pool.tile([128, B], F32)
        i_eQ = [None, None]
        for h in range(2):
            sl = slice(h * H, (h + 1) * H)
            # union is an integer >= 1 for this input family (a row with
            # zero positive concepts has probability ~ (2/3)^256)
            nc.vector.reciprocal_approx_fast(urec[:, sl], pU[h][:])
            nc.vector.scalar_tensor_tensor(
                QC[0:BLK, sl], pI[h][:], 5.0, urec[:, sl], ALU.mult, ALU.mult
            )
        for h in range(2):
            sl = slice(h * H, (h + 1) * H)
            i_eQ[h] = nc.scalar.activation(
                X[:, sl], QC[:, sl], AF.Exp, accum_out=V[:, 5 + h : 6 + h]
            ).ins
        for h in range(2):
            sl = slice(h * H, (h + 1) * H)
            nc.vector.tensor_copy(X[BLK:128, sl], X[0:BLK, sl])
            nc.vector.scalar_tensor_tensor(
                scrapP[:, sl], X[:, sl], 1.0, PT[:, sl], ALU.bypass, ALU.mult,
                accum_out=V[:, h : h + 1],
            )
            nc.vector.scalar_tensor_tensor(
                scrapQ[:, sl], QC[:, sl], 1.0, X[:, sl], ALU.bypass, ALU.mult,
                accum_out=V[:, 2 + h : 3 + h],
            )

        # exp of [img; txt] only feeds its row-sum Z; keep it last on ACT
        ePs = pool.tile([128, B], BF16)  # scrap
        i_ept = nc.scalar.activation(ePs[:], PT[:], AF.Exp, accum_out=V[:, 4:5]).ins

        # pin the ACT queue order (Tile otherwise reorders by readiness)
        order = [i_dummy, i_esx, i_eln, i_eQ[0], i_eQ[1], i_ept]
        for a, b_ in zip(order[1:], order[:-1]):
            add_dep_helper(a, b_, False, "act-order")

        nc.sync.dma_start(vout[:], V[:])

    nc.compile()
    return nc


def _pack_T(mat: np.ndarray) -> np.ndarray:
    """[256, W] -> [128, 2, W] with [p, two, j] = mat[two*128+p, j]."""
    w = mat.shape[1]
    return np.ascontiguousarray(mat.reshape(2, 128, w).transpose(1, 0, 2))


def make_in_maps(inputs):
    li = np.asarray(inputs["logits_per_image"], dtype=np.float32)
    lt = np.asarray(inputs["logits_per_text"], dtype=np.float32)
    cl = np.asarray(inputs["concepts_logits"], dtype=np.float32)
    cis = np.asarray(inputs["concepts_image_similarity"], dtype=np.float32)
    mc = np.asarray(inputs["medical_concepts"])

    c = (mc == 1).astype(np.float32)                  # [512, 256]
    s = ((mc != -1) * (1 - 2 * (mc == 1))).astype(np.float32)
    cT = _pack_T(np.ascontiguousarray(c.T))           # [128, 2, 512]
    omT = _pack_T(np.ascontiguousarray((1.0 - c).T))  # [128, 2, 512]

    in_maps = []
    for k in range(NCORES):
        sl = slice(k * BLK, (k + 1) * BLK)
        cpa = np.concatenate([cT[:, :, 0:H], omT[:, :, sl], cT[:, :, sl]], axis=2)
        cpb = cT[:, :, H:B]
        # lower half of qcin: [cl | s] as bf16 bytes viewed f32, zero-padded
        cls16 = np.concatenate([cl[sl], s[sl]], axis=1).astype(ml_dtypes.bfloat16)
        low = np.zeros((BLK, B), dtype=np.float32)
        low[:, 0:C] = cls16.view(np.float32)
        in_maps.append({
            "cpa": np.ascontiguousarray(cpa.reshape(128, -1)).astype(ml_dtypes.float8_e4m3),
            "cpb": np.ascontiguousarray(cpb.reshape(128, -1)).astype(ml_dtypes.float8_e4m3),
            "pt": np.concatenate([li[sl], lt[sl]], axis=0).astype(ml_dtypes.bfloat16),
            "qcin": np.concatenate([low, cis[sl]], axis=0),
        })
    return in_maps


def combine_partials(parts, mc) -> np.ndarray:
    """Host fp64 combine of per-row raw stats from the 8 cores."""
    v = np.concatenate([np.asarray(p, dtype=np.float64) for p in parts], axis=0)
    v = v.reshape(NCORES, 128, NST)
    lo, hi = v[:, 0:BLK, :], v[:, BLK:128, :]
    dot_img, dot_txt = lo[..., 0] + lo[..., 1], hi[..., 0] + hi[..., 1]
    dot_h5, dot_cis = lo[..., 2] + lo[..., 3], hi[..., 2] + hi[..., 3]
    z_img, z_txt = lo[..., 4], hi[..., 4]
    z_sim, z_cis = lo[..., 5] + lo[..., 6], hi[..., 5] + hi[..., 6]
    bce_rows = lo[..., 7]

    Hrow = dot_h5 / z_sim - np.log(z_sim)
    a_img = dot_img / z_sim - np.log(z_img)
    a_txt = dot_txt / z_sim - np.log(z_txt)
    a_cis = dot_cis / z_sim - np.log(z_cis)

    clip = np.sum(2.0 * Hrow - a_img - a_txt) / (2.0 * B)
    csim = np.sum(Hrow - a_cis) / B

    n_masked = float(np.sum(mc == -1))
    mask_sum = float(mc.size - n_masked)
    bce_sum = float(np.sum(bce_rows)) - LN2 * n_masked
    conc = bce_sum / (mask_sum + 1e-8)

    total = clip + 0.2 * conc + 0.2 * csim
    return np.asarray(total, dtype=np.float32)


def _run(inputs, trace=False):
    if "nc" not in _CACHE:
        _CACHE["nc"] = build_nc()
    nc = _CACHE["nc"]
    res = bass_utils.run_bass_kernel_spmd(
        nc, make_in_maps(inputs), core_ids=list(range(NCORES)), trace=trace
    )
    parts = [res.results[k]["vout"] for k in range(NCORES)]
    mc = np.asarray(inputs["medical_concepts"])
    return combine_partials(parts, mc), res


def kernel(**inputs) -> np.ndarray:
    out, _ = _run(inputs, trace=bool(int(os.environ.get("KERNEL_TRACE", "0"))))
    return out



# revision 1
# speedup vs baseline: 1.6372x; 1.6372x over previous
"""CCALoss (soft-contrastive CLIP + masked BCE + concept-sim KL) on 8 trn2 cores.

Math: with c = (mc==1) binary, jaccard inter = c@cT, union = r_i + r_j -
inter. All three PE contractions (r_j - inter via (1-c)@c, +r_i via
c@ones, inter via c@c) run as fp8 DoubleRow matmuls (K=256 in one pass).
sim5 = 5*inter/union via DVE recip + a fused scalar_tensor_tensor; the
four per-row dots sum_j e^(5sim)*X for X in {img, txt, 5sim, cis} are
fused STT ops with row accumulators. BCE uses the sign trick
bce = ln(1+e^(s*x)), s = mask*(1-2t) in {-1,0,+1}; masked entries each
contribute ln2, subtracted exactly on host. The device ships per-row raw
stats V[128,8] (dots, softmax partition sums Z, bce row sums); the host
does every 1/Z, ln and the final scalar combine in fp64.

Schedule: inputs split into five DMAs on three queues so each consumer's
bytes land just before it runs (sync: fp8 pack in two column chunks so
the a-half matmuls start early; scalar: [cls|cis] halves; gpsimd:
logits). A dependency-free dummy Ln leads the ACT queue so both
ACT_TABLE_LOADs run under the DMA shadow; explicit order deps pin the
ACT sequence exp_sx -> ln -> expQ_a -> expQ_b -> exp_pt. The
union->recip->5sim->exp->dots chain is pipelined in column halves with
per-half accumulator columns in V summed on host.
"""

import os
import numpy as np
import types
from contextlib import ExitStack

import ml_dtypes

import bass_rust as _bass_rust
import concourse.bacc as bacc
import concourse.mybir as mybir
import concourse.tile as tile
from concourse.tile_rust import add_dep_helper
from concourse import bass_utils
from concourse.hw_specs import get_activation_tables

F32 = mybir.dt.float32
BF16 = mybir.dt.bfloat16
FP8 = mybir.dt.float8e4
U32 = mybir.dt.uint32
AF = mybir.ActivationFunctionType
ALU = mybir.AluOpType
AX = mybir.AxisListType

B = 512          # batch
C = 256          # concepts
H = 256          # column half for the pipelined tail
NCORES = 8
BLK = B // NCORES  # 64 rows per core
NST = 8          # stat columns in V

# V column layout ([128, NST]; rows 0:64 / 64:128 hold different stats)
# 0/1: dot e*[img;txt] halves a/b   2/3: dot e*[5sim;cis] halves a/b
# 4: Z of [img;txt]                 5/6: Z of [5sim;cis] halves a/b
# 7: lower only, sum_j ln(1+e^(s*x))
_CACHE = {}

LN2 = float(np.log(2.0))
FP8_ONE_X4 = 0x38383838  # four fp8e4m3 1.0 bytes per uint32


def _patched_act_table_loads(self):
    """Force exp+ln onto the single natural_log_exp_and_others set.

    Same contract as Bacc.insert_act_table_loads: the list index must
    stay aligned with act_info.json (walrus remaps index -> runtime id),
    so sets keep their positions and only lose exp/ln membership.
    """
    has_activation = any(
        isinstance(i, mybir.InstActivation)
        for b in self.main_func.blocks
        for i in b.instructions
    )
    if not has_activation:
        return
    keep = "natural_log_exp_and_others"
    both = {AF.Exp, AF.Ln}
    tables = [
        (name, set(fns) if name == keep else set(fns) - both)
        for name, fns in get_activation_tables(self.m.arch).items()
    ]
    _bass_rust.insert_act_table_loads(self, tables)


def build_nc():
    nc = bacc.Bacc(
        "TRN2", target_bir_lowering=False, debug=False, num_devices=NCORES
    )
    nc.insert_act_table_loads = types.MethodType(_patched_act_table_loads, nc)

    # fp8 packs: [p, two, j] = c^T; A carries batch cols 0:256 plus the
    # stationary blocks ((1-c)^T blk, c^T blk), B carries cols 256:512
    cpa_in = nc.dram_tensor("cpa", [128, 2 * (H + 2 * BLK)], FP8, kind="ExternalInput").ap()
    cpb_in = nc.dram_tensor("cpb", [128, 2 * H], FP8, kind="ExternalInput").ap()
    # [128,512] bf16: rows 0:64 = logits_per_image blk, 64:128 = logits_per_text blk
    pt_in = nc.dram_tensor("pt", [128, B], BF16, kind="ExternalInput").ap()
    # [128,512] f32: rows 0:64 = [concepts_logits blk | s]-as-bf16-bytes (+pad),
    # rows 64:128 = concepts_image_similarity blk
    qc_in = nc.dram_tensor("qcin", [128, B], F32, kind="ExternalInput").ap()
    vout = nc.dram_tensor("vout", [128, NST], F32, kind="ExternalOutput").ap()

    with tile.TileContext(nc) as tc, ExitStack() as ctx:
        pool = ctx.enter_context(tc.tile_pool(name="main", bufs=1))
        psum = ctx.enter_context(tc.tile_pool(name="psum", bufs=1, space="PSUM"))

        cpa = pool.tile([128, 2, H + 2 * BLK], FP8)
        cpb = pool.tile([128, 2, H], FP8)
        ones32 = pool.tile([128, 2 * B // 4], U32)
        PT = pool.tile([128, B], BF16)
        QC = pool.tile([128, B], F32)  # 0:64 = cls bytes then 5sim; 64:128 = cis
        V = pool.tile([128, NST], F32)

        ones = ones32[:].bitcast(FP8).rearrange("p (two w) -> p two w", two=2)
        cls = QC[0:BLK, 0:C].bitcast(BF16)  # [64, 512] bf16: [cl | s]

        # ---- DMA issue spread across engine queues (parallel at t=0) ----
        nc.vector.memset(ones32[:], FP8_ONE_X4)
        nc.vector.memset(V[:], 0.0)
        nc.sync.dma_start(cpa[:], cpa_in[:].rearrange("p (two w) -> p two w", two=2))
        nc.sync.dma_start(cpb[:], cpb_in[:].rearrange("p (two w) -> p two w", two=2))
        nc.scalar.dma_start(QC[:, 0:H], qc_in[:, 0:H])
        nc.scalar.dma_start(QC[:, H:B], qc_in[:, H:B])
        nc.gpsimd.dma_start(PT[:], pt_in[:])

        cfa = cpa[:, :, 0:H]
        onemcb = cpa[:, :, H : H + BLK]
        cblkb = cpa[:, :, H + BLK : H + 2 * BLK]
        cfb = cpb[:, :, 0:H]
        DR = mybir.MatmulPerfMode.DoubleRow

        # ---- jaccard contractions: union and inter, fp8 DoubleRow ----
        pU = [psum.tile([BLK, H], F32, name=f"pU{h}") for h in range(2)]
        pI = [psum.tile([BLK, H], F32, name=f"pI{h}") for h in range(2)]
        for h, cf in enumerate((cfa, cfb)):
            on = ones[:, :, h * H : (h + 1) * H]
            nc.tensor.matmul(pU[h][:], onemcb, cf, start=True, stop=False, perf_mode=DR)
            nc.tensor.matmul(pU[h][:], cblkb, on, start=False, stop=True, perf_mode=DR)
            nc.tensor.matmul(pI[h][:], cblkb, cf, start=True, stop=True, perf_mode=DR)

        # ---- dummy ln: anchors the ACT table load at t~0 (no data deps) ----
        dummy = pool.tile([1, 1], F32)
        i_dummy = nc.scalar.activation(dummy[:], V[0:1, 0:1], AF.Ln, bias=1.0).ins

        # ---- BCE: sx = s*x, then ln(1 + e^sx) row-summed by the ACT accum ----
        sxr = pool.tile([BLK, C], BF16)
        nc.vector.tensor_tensor(sxr[:], cls[:, C : 2 * C], cls[:, 0:C], ALU.mult)
        bexp = pool.tile([BLK, C], BF16)
        i_esx = nc.scalar.activation(bexp[:], sxr[:], AF.Exp).ins
        bln = pool.tile([BLK, C], BF16)  # scrap; accum is the payload
        i_eln = nc.scalar.activation(
            bln[:], bexp[:], AF.Ln, bias=1.0, accum_out=V[0:BLK, 7:8]
        ).ins

        # ---- pipelined halves: recip -> 5sim -> exp -> dup -> dots ----
        urec = pool.tile([BLK, B], F32)
        X = pool.tile([128, B], BF16)       # e^[5sim; cis]
        scrapP = pool.tile([128, B], BF16)
        scrapQ = pool.tile([128, B], F32)
        i_eQ = [None, None]
        for h in range(2):
            sl = slice(h * H, (h + 1) * H)
            # union is an integer >= 1 for this input family (a row with
            # zero positive concepts has probability ~ (2/3)^256)
            nc.vector.reciprocal_approx_fast(urec[:, sl], pU[h][:])
            nc.vector.scalar_tensor_tensor(
                QC[0:BLK, sl], pI[h][:], 5.0, urec[:, sl], ALU.mult, ALU.mult
            )
        for h in range(2):
            sl = slice(h * H, (h + 1) * H)
            i_eQ[h] = nc.scalar.activation(
                X[:, sl], QC[:, sl], AF.Exp, accum_out=V[:, 5 + h : 6 + h]
            ).ins
        for h in range(2):
            sl = slice(h * H, (h + 1) * H)
            nc.vector.tensor_copy(X[BLK:128, sl], X[0:BLK, sl])
            nc.vector.scalar_tensor_tensor(
                scrapP[:, sl], X[:, sl], 1.0, PT[:, sl], ALU.bypass, ALU.mult,
                accum_out=V[:, h : h + 1],
            )
            nc.vector.scalar_tensor_tensor(
                scrapQ[:, sl], QC[:, sl], 1.0, X[:, sl], ALU.bypass, ALU.mult,
                accum_out=V[:, 2 + h : 3 + h],
            )

        # exp of [img; txt] only feeds its row-sum Z; keep it last on ACT
        ePs = pool.tile([128, B], BF16)  # scrap
        i_ept = nc.scalar.activation(ePs[:], PT[:], AF.Exp, accum_out=V[:, 4:5]).ins

        # pin the ACT queue order (Tile otherwise reorders by readiness)
        order = [i_dummy, i_esx, i_eln, i_eQ[0], i_eQ[1], i_ept]
        for a, b_ in zip(order[1:], order[:-1]):
            add_dep_helper(a, b_, False, "act-order")

        nc.sync.dma_start(vout[:], V[:])

    nc.compile()
    return nc


def _pack_T(mat: np.ndarray) -> np.ndarray:
    """[256, W] -> [128, 2, W] with [p, two, j] = mat[two*128+p, j]."""
    w = mat.shape[1]
    return np.ascontiguousarray(mat.reshape(2, 128, w).transpose(1, 0, 2))


def make_in_maps(inputs):
    li = np.asarray(inputs["logits_per_image"], dtype=np.float32)
    lt = np.asarray(inputs["logits_per_text"], dtype=np.float32)
    cl = np.asarray(inputs["concepts_logits"], dtype=np.float32)
    cis = np.asarray(inputs["concepts_image_similarity"], dtype=np.float32)
    mc = np.asarray(inputs["medical_concepts"])

    c = (mc == 1).astype(np.float32)                  # [512, 256]
    s = ((mc != -1) * (1 - 2 * (mc == 1))).astype(np.float32)
    cT = _pack_T(np.ascontiguousarray(c.T))           # [128, 2, 512]
    omT = _pack_T(np.ascontiguousarray((1.0 - c).T))  # [128, 2, 512]

    in_maps = []
    for k in range(NCORES):
        sl = slice(k * BLK, (k + 1) * BLK)
        cpa = np.concatenate([cT[:, :, 0:H], omT[:, :, sl], cT[:, :, sl]], axis=2)
        cpb = cT[:, :, H:B]
        # lower half of qcin: [cl | s] as bf16 bytes viewed f32, zero-padded
        cls16 = np.concatenate([cl[sl], s[sl]], axis=1).astype(ml_dtypes.bfloat16)
        low = np.zeros((BLK, B), dtype=np.float32)
        low[:, 0:C] = cls16.view(np.float32)
        in_maps.append({
            "cpa": np.ascontiguousarray(cpa.reshape(128, -1)).astype(ml_dtypes.float8_e4m3),
            "cpb": np.ascontiguousarray(cpb.reshape(128, -1)).astype(ml_dtypes.float8_e4m3),
            "pt": np.concatenate([li[sl], lt[sl]], axis=0).astype(ml_dtypes.bfloat16),
            "qcin": np.concatenate([low, cis[sl]], axis=0),
        })
    return in_maps


def combine_partials(parts, mc) -> np.ndarray:
    """Host fp64 combine of per-row raw stats from the 8 cores."""
    v = np.concatenate([np.asarray(p, dtype=np.float64) for p in parts], axis=0)
    v = v.reshape(NCORES, 128, NST)
    lo, hi = v[:, 0:BLK, :], v[:, BLK:128, :]
    dot_img, dot_txt = lo[..., 0] + lo[..., 1], hi[..., 0] + hi[..., 1]
    dot_h5, dot_cis = lo[..., 2] + lo[..., 3], hi[..., 2] + hi[..., 3]
    z_img, z_txt = lo[..., 4], hi[..., 4]
    z_sim, z_cis = lo[..., 5] + lo[..., 6], hi[..., 5] + hi[..., 6]
    bce_rows = lo[..., 7]

    Hrow = dot_h5 / z_sim - np.log(z_sim)
    a_img = dot_img / z_sim - np.log(z_img)
    a_txt = dot_txt / z_sim - np.log(z_txt)
    a_cis = dot_cis / z_sim - np.log(z_cis)

    clip = np.sum(2.0 * Hrow - a_img - a_txt) / (2.0 * B)
    csim = np.sum(Hrow - a_cis) / B

    n_masked = float(np.sum(mc == -1))
    mask_sum = float(mc.size - n_masked)
    bce_sum = float(np.sum(bce_rows)) - LN2 * n_masked
    conc = bce_sum / (mask_sum + 1e-8)

    total = clip + 0.2 * conc + 0.2 * csim
    return np.asarray(total, dtype=np.float32)


def _run(inputs, trace=False):
    if "nc" not in _CACHE:
        _CACHE["nc"] = build_nc()
    nc = _CACHE["nc"]
    res = bass_utils.run_bass_kernel_spmd(
        nc, make_in_maps(inputs), core_ids=list(range(NCORES)), trace=trace
    )
    parts = [res.results[k]["vout"] for k in range(NCORES)]
    mc = np.asarray(inputs["medical_concepts"])
    return combine_partials(parts, mc), res


def kernel(**inputs) -> np.ndarray:
    out, _ = _run(inputs, trace=bool(int(os.environ.get("KERNEL_TRACE", "0"))))
    return out

